# revision 35
# baseline (speedup 1.0000x reference)
"""Gaussian KDE (brute-force, bandwidth^2 = 1) on 8 Trainium2 NeuronCores.

Math:
    out_i = log( sum_j w_j * exp(-||x_i - y_j||^2/2) ) - (d/2) log(2pi) - log(sum_j w_j)
          = log( sum_j exp(x_i . y_j + b_j) ) - ||x_i||^2/2 - consts
    with b_j = log(w_j) - ||y_j||^2/2.

Queries sharded 8 ways (512/core, 4 PSUM-partition tiles). Per core:
    - scores: K=35 bf16 matmuls, stationary = query tile [35, 128], moving =
      train slices. Operands are pre-scaled so PSUM holds C1*s + C2b/32
      (C1 = 4/ln2, C2b the bf16 Schraudolph bias): x rows = C1*x dims plus
      three 4.0 rows; y rows = y dims + (C1/4)*b hi + lo + C2b/128 row.
      K=35 <= 64, so consecutive matmuls alternate PE row groups via
      tile_position (0,0)/(64,0) and run pairwise-concurrently.
    - exp+sum, two unit types balanced by a credit scheduler across engines:
      * ACT unit [128, 1536] (3 banks): table-exp in place with
        scale=1/C1, bias=-C2b/(32*C1), free-dim sum fused via accum_out.
      * DVE unit [128, 512] (1 bank): tensor_scalar (mult 32, max 0) whose
        int16 result IS the bf16 bit pattern of exp (Schraudolph; the max
        makes int16 wrap impossible), then tensor_reduce of the bitcast.
    - final: per query tile reduce partials, ln, subtract per-query const.
"""

import numpy as np
import ml_dtypes

_Q, _N, _D = 4096, 65536, 32
_NCORES = 8
_QSHARD = _Q // _NCORES          # 512 queries per core
_K = 34                          # 32 dims + bias hi/lo (incl C2 const)
_QT = 4                          # query tiles per core

_BF16 = ml_dtypes.bfloat16

_C1 = 4.0 / float(np.log(2.0))


def _c2b():
    f = (np.arange(100000, dtype=np.float64) + 0.5) / 100000.0
    m0 = np.mean((1.0 + f) * 2.0 ** (-f))
    m1 = np.mean(2.0 ** (-f))
    delta = (m0 - 1.0) / m1
    return float(127 * 128 - delta * 128)


_C2B = _c2b()

# per query tile: trains covered by ACT units (1536 each) and DVE units (512)
_NA = 30                         # 30 * 1536 = 46080
_ND = 38                         # 38 * 512  = 19456 ; total 65536
_ACT_NS = 1660.0                 # measured per ACT unit
_DVE_NS = 1294.0                 # measured per DVE unit (incl half reduce)

_prog_cache: dict = {}


def _unit_schedule():
    """Credit-scheduled unit type sequence (shared by all 4 query tiles)."""
    seq = []
    na, nd, ta, td = 0, 0, 0.0, -1300.0   # one D unit head start
    while na < _NA or nd < _ND:
        if nd >= _ND or (na < _NA and ta <= td):
            seq.append('A')
            na += 1
            ta += _ACT_NS
        else:
            seq.append('D')
            nd += 1
            td += _DVE_NS
    return seq


def _build_program(n_trains: int):
    import concourse.bass as bass
    import concourse.tile as tile
    from concourse import bacc, mybir

    f32 = mybir.dt.float32
    bf16 = mybir.dt.bfloat16
    i16 = mybir.dt.int16

    nc = bacc.Bacc("TRN2", target_bir_lowering=False, debug=False,
                   num_devices=_NCORES)

    y_d = nc.dram_tensor("yext", [_K, n_trains], bf16, kind="ExternalInput")
    x_d = nc.dram_tensor("xext", [_K, _QSHARD], bf16, kind="ExternalInput")
    out_d = nc.dram_tensor("out", [128, _QT], f32, kind="ExternalOutput")

    seq = _unit_schedule()

    with tile.TileContext(nc) as tc:
        with (
            tc.tile_pool(name="const", bufs=1) as cpool,
            tc.tile_pool(name="q16", bufs=8) as qpool,
            tc.tile_pool(name="small", bufs=2) as spool,
            tc.tile_pool(name="ps", bufs=1, space="PSUM") as ppool,
        ):
            xsb = cpool.tile([128, _QSHARD], bf16)
            nc.sync.dma_start(xsb[0:_K, :], x_d[:])
            nc.sync.dma_start(xsb[64:64 + _K, :], x_d[:])
            bias_sb = cpool.tile([128, 1], f32)
            nc.vector.memset(bias_sb[:], -_C2B / (32.0 * _C1))

            # y resident in SBUF, both row-group strips, 8 DMA pieces each
            ysb = cpool.tile([128, n_trains], bf16)
            pieces = [4096] + [2048] * 30
            off = 0
            for w in pieces:
                nc.sync.dma_start(ysb[0:_K, off:off + w],
                                  y_d[:, off:off + w])
                nc.sync.dma_start(ysb[64:64 + _K, off:off + w],
                                  y_d[:, off:off + w])
                off += w

            NCQ = _NA + (_ND + 1) // 2
            sall = cpool.tile([128, NCQ * _QT], f32)

            # PSUM: A units double-buffered at [0:1536],[1536:3072];
            # D units at [3072:3584],[3584:4096]
            ps = ppool.tile([128, 8 * 512], f32)

            rg_par = [0]

            def score_mm(qt, dst, t0, width):
                for j in range(width // 512):
                    rg = 64 * (rg_par[0] & 1)
                    rg_par[0] += 1
                    nc.tensor.matmul(
                        out=ps[:, dst + j * 512: dst + (j + 1) * 512],
                        lhsT=xsb[rg:rg + _K, qt * 128:(qt + 1) * 128],
                        rhs=ysb[rg:rg + _K, t0 + j * 512: t0 + (j + 1) * 512],
                        start=True, stop=True,
                        tile_position=(rg, 0),
                    )

            pcol = [0] * _QT
            gen = {'A': 0, 'D': 0}
            cur = [0] * _QT
            half = [None] * _QT       # pending first half of a D pair
            for typ in seq:
                for qt in range(_QT):
                    t0 = cur[qt]
                    if typ == 'A':
                        dst = 1536 * (gen['A'] & 1)
                        gen['A'] += 1
                        score_mm(qt, dst, t0, 1536)
                        c = qt * NCQ + pcol[qt]
                        pcol[qt] += 1
                        nc.scalar.activation(
                            ps[:, dst:dst + 1536], ps[:, dst:dst + 1536],
                            mybir.ActivationFunctionType.Exp,
                            bias=bias_sb[:], scale=1.0 / _C1,
                            accum_out=sall[:, c:c + 1])
                        cur[qt] = t0 + 1536
                    else:
                        dst = 3072 + 512 * (gen['D'] & 1)
                        gen['D'] += 1
                        score_mm(qt, dst, t0, 512)
                        if half[qt] is None:
                            q16 = qpool.tile([128, 2, 512], i16)
                            nc.vector.tensor_scalar(
                                q16[:, 0, :], ps[:, dst:dst + 512], 32.0, 0.0,
                                mybir.AluOpType.mult, mybir.AluOpType.max)
                            half[qt] = q16
                        else:
                            q16 = half[qt]
                            half[qt] = None
                            nc.vector.tensor_scalar(
                                q16[:, 1, :], ps[:, dst:dst + 512], 32.0, 0.0,
                                mybir.AluOpType.mult, mybir.AluOpType.max)
                            c = qt * NCQ + pcol[qt]
                            pcol[qt] += 1
                            nc.vector.tensor_reduce(
                                sall[:, c:c + 1], q16[:].bitcast(bf16),
                                axis=mybir.AxisListType.XY,
                                op=mybir.AluOpType.add)
                        cur[qt] = t0 + 512

            for qt in range(_QT):
                if half[qt] is not None:
                    q16 = half[qt]
                    c = qt * NCQ + pcol[qt]
                    pcol[qt] += 1
                    nc.vector.tensor_reduce(
                        sall[:, c:c + 1], q16[:, 0, :].bitcast(bf16),
                        axis=mybir.AxisListType.X, op=mybir.AluOpType.add)

            fin = spool.tile([128, _QT], f32)
            for qt in range(_QT):
                nc.vector.tensor_reduce(
                    fin[:, qt:qt + 1], sall[:, qt * NCQ:qt * NCQ + pcol[qt]],
                    axis=mybir.AxisListType.X, op=mybir.AluOpType.add)
            nc.sync.dma_start(out_d[:], fin[:])

    nc.compile()
    return nc


def _get_program(n_trains: int):
    if n_trains not in _prog_cache:
        _prog_cache[n_trains] = _build_program(n_trains)
    return _prog_cache[n_trains]


def _prep_inputs(X, X_train, sample_weight):
    X = np.ascontiguousarray(np.asarray(X, dtype=np.float32))
    Y = np.ascontiguousarray(np.asarray(X_train, dtype=np.float32))
    w = np.ascontiguousarray(np.asarray(sample_weight, dtype=np.float32))
    n = Y.shape[0]

    w64 = w.astype(np.float64)
    b64 = np.log(np.maximum(w64, 1e-300)) - 0.5 * np.sum(
        Y.astype(np.float64) ** 2, axis=1)
    b64 = np.clip(b64, -35.0, None)
    cb64 = (_C1 * b64 + _C2B / 32.0) / 4.0
    bhi = cb64.astype(np.float32).astype(_BF16)
    blo = (cb64 - bhi.astype(np.float64)).astype(np.float32).astype(_BF16)

    yext = np.empty((_K, n), dtype=_BF16)
    yext[0:32] = Y.astype(_BF16).T
    yext[32] = bhi
    yext[33] = blo

    const = 0.5 * _D * np.log(2.0 * np.pi) + np.log(np.sum(w64))
    xsq = np.sum(X.astype(np.float64) ** 2, axis=1)
    dv_all = (0.5 * xsq + const).astype(np.float32)

    in_maps = []
    dvs = []
    for c in range(_NCORES):
        sl = slice(c * _QSHARD, (c + 1) * _QSHARD)
        xq = X[sl]
        xext = np.empty((_K, _QSHARD), dtype=_BF16)
        xext[0:32] = (_C1 * xq.astype(np.float64)).astype(_BF16).T
        xext[32] = np.full(_QSHARD, 4.0, dtype=_BF16)
        xext[33] = np.full(_QSHARD, 4.0, dtype=_BF16)
        dv = np.ascontiguousarray(dv_all[sl].reshape(_QT, 128).T)
        in_maps.append({"yext": yext, "xext": xext})
        dvs.append(dv_all[sl].astype(np.float64))
    return in_maps, dvs


def _gather(results, dvs):
    out = np.empty(_Q, dtype=np.float32)
    for c in range(_NCORES):
        tot = results[c]["out"].T.reshape(_QSHARD).astype(np.float64)
        out[c * _QSHARD:(c + 1) * _QSHARD] = np.log(tot) - dvs[c]
    return out


def kernel(X, X_train, sample_weight, _want_timing=False):
    from concourse.bass_utils import run_bass_kernel_spmd

    nc = _get_program(_N)
    in_maps, dvs = _prep_inputs(X, X_train, sample_weight)
    kres = run_bass_kernel_spmd(
        nc, in_maps, core_ids=list(range(_NCORES)),
        trace=bool(_want_timing),
    )
    out = _gather(kres.results, dvs)
    if _want_timing:
        return out, kres
    return out


# revision 36
# speedup vs baseline: 1.0201x; 1.0201x over previous
"""Gaussian KDE (brute-force, bandwidth^2 = 1) on 8 Trainium2 NeuronCores.

Math:
    out_i = log( sum_j w_j * exp(-||x_i - y_j||^2/2) ) - (d/2) log(2pi) - log(sum_j w_j)
          = log( sum_j exp(x_i . y_j + b_j) ) - ||x_i||^2/2 - consts
    with b_j = log(w_j) - ||y_j||^2/2.

Queries sharded 8 ways (512/core, 4 PSUM-partition tiles). Per core:
    - scores: K=35 bf16 matmuls, stationary = query tile [35, 128], moving =
      train slices. Operands are pre-scaled so PSUM holds C1*s + C2b/32
      (C1 = 4/ln2, C2b the bf16 Schraudolph bias): x rows = C1*x dims plus
      three 4.0 rows; y rows = y dims + (C1/4)*b hi + lo + C2b/128 row.
      K=35 <= 64, so consecutive matmuls alternate PE row groups via
      tile_position (0,0)/(64,0) and run pairwise-concurrently.
    - exp+sum, two unit types balanced by a credit scheduler across engines:
      * ACT unit [128, 1536] (3 banks): table-exp in place with
        scale=1/C1, bias=-C2b/(32*C1), free-dim sum fused via accum_out.
      * DVE unit [128, 512] (1 bank): tensor_scalar (mult 32, max 0) whose
        int16 result IS the bf16 bit pattern of exp (Schraudolph; the max
        makes int16 wrap impossible), then tensor_reduce of the bitcast.
    - final: per query tile reduce partials, ln, subtract per-query const.
"""

import numpy as np
import ml_dtypes

_Q, _N, _D = 4096, 65536, 32
_NCORES = 8
_QSHARD = _Q // _NCORES          # 512 queries per core
_K = 34                          # 32 dims + bias hi/lo (incl C2 const)
_QT = 4                          # query tiles per core

_BF16 = ml_dtypes.bfloat16

_C1 = 4.0 / float(np.log(2.0))


def _c2b():
    f = (np.arange(100000, dtype=np.float64) + 0.5) / 100000.0
    m0 = np.mean((1.0 + f) * 2.0 ** (-f))
    m1 = np.mean(2.0 ** (-f))
    delta = (m0 - 1.0) / m1
    return float(127 * 128 - delta * 128)


_C2B = _c2b()

# per query tile: trains covered by ACT units (1536 each) and DVE units (512)
_NA = 30                         # 30 * 1536 = 46080
_ND = 38                         # 38 * 512  = 19456 ; total 65536
_ACT_NS = 1660.0                 # measured per ACT unit
_DVE_NS = 1294.0                 # measured per DVE unit (incl half reduce)

_prog_cache: dict = {}


def _unit_schedule():
    """Credit-scheduled unit type sequence (shared by all 4 query tiles)."""
    seq = []
    na, nd, ta, td = 0, 0, 0.0, -1300.0   # one D unit head start
    while na < _NA or nd < _ND:
        if nd >= _ND or (na < _NA and ta <= td):
            seq.append('A')
            na += 1
            ta += _ACT_NS
        else:
            seq.append('D')
            nd += 1
            td += _DVE_NS
    return seq


def _build_program(n_trains: int):
    import concourse.bass as bass
    import concourse.tile as tile
    from concourse import bacc, mybir

    f32 = mybir.dt.float32
    bf16 = mybir.dt.bfloat16
    i16 = mybir.dt.int16

    nc = bacc.Bacc("TRN2", target_bir_lowering=False, debug=False,
                   num_devices=_NCORES)

    y_d = nc.dram_tensor("yext", [_K, n_trains], bf16, kind="ExternalInput")
    x_d = nc.dram_tensor("xext", [_K, _QSHARD], bf16, kind="ExternalInput")
    out_d = nc.dram_tensor("out", [128, _QT], f32, kind="ExternalOutput")

    seq = _unit_schedule()

    with tile.TileContext(nc) as tc:
        with (
            tc.tile_pool(name="const", bufs=1) as cpool,
            tc.tile_pool(name="q16", bufs=8) as qpool,
            tc.tile_pool(name="small", bufs=2) as spool,
            tc.tile_pool(name="ps", bufs=1, space="PSUM") as ppool,
        ):
            xsb = cpool.tile([128, _QSHARD], bf16)
            nc.sync.dma_start(xsb[0:_K, :], x_d[:])
            nc.sync.dma_start(xsb[64:64 + _K, :], x_d[:])
            bias_sb = cpool.tile([128, 1], f32)
            nc.vector.memset(bias_sb[:], -_C2B / (32.0 * _C1))

            # y resident in SBUF, both row-group strips, 8 DMA pieces each
            ysb = cpool.tile([128, n_trains], bf16)
            pieces = [1024, 1024] + [2048] * 31
            off = 0
            for w in pieces:
                nc.sync.dma_start(ysb[0:_K, off:off + w],
                                  y_d[:, off:off + w])
                nc.sync.dma_start(ysb[64:64 + _K, off:off + w],
                                  y_d[:, off:off + w])
                off += w

            NCQ = _NA + (_ND + 1) // 2
            sall = cpool.tile([128, NCQ * _QT], f32)

            # PSUM: A units double-buffered at [0:1536],[1536:3072];
            # D units at [3072:3584],[3584:4096]
            ps = ppool.tile([128, 8 * 512], f32)

            rg_par = [0]

            def score_mm(qt, dst, t0, width):
                for j in range(width // 512):
                    rg = 64 * (rg_par[0] & 1)
                    rg_par[0] += 1
                    nc.tensor.matmul(
                        out=ps[:, dst + j * 512: dst + (j + 1) * 512],
                        lhsT=xsb[rg:rg + _K, qt * 128:(qt + 1) * 128],
                        rhs=ysb[rg:rg + _K, t0 + j * 512: t0 + (j + 1) * 512],
                        start=True, stop=True,
                        tile_position=(rg, 0),
                    )

            pcol = [0] * _QT
            gen = {'A': 0, 'D': 0}
            cur = [0] * _QT
            half = [None] * _QT       # pending first half of a D pair
            for typ in seq:
                for qt in range(_QT):
                    t0 = cur[qt]
                    if typ == 'A':
                        dst = 1536 * (gen['A'] & 1)
                        gen['A'] += 1
                        score_mm(qt, dst, t0, 1536)
                        c = qt * NCQ + pcol[qt]
                        pcol[qt] += 1
                        nc.scalar.activation(
                            ps[:, dst:dst + 1536], ps[:, dst:dst + 1536],
                            mybir.ActivationFunctionType.Exp,
                            bias=bias_sb[:], scale=1.0 / _C1,
                            accum_out=sall[:, c:c + 1])
                        cur[qt] = t0 + 1536
                    else:
                        dst = 3072 + 512 * (gen['D'] & 1)
                        gen['D'] += 1
                        score_mm(qt, dst, t0, 512)
                        if half[qt] is None:
                            q16 = qpool.tile([128, 2, 512], i16)
                            nc.vector.tensor_scalar(
                                q16[:, 0, :], ps[:, dst:dst + 512], 32.0, 0.0,
                                mybir.AluOpType.mult, mybir.AluOpType.max)
                            half[qt] = q16
                        else:
                            q16 = half[qt]
                            half[qt] = None
                            nc.vector.tensor_scalar(
                                q16[:, 1, :], ps[:, dst:dst + 512], 32.0, 0.0,
                                mybir.AluOpType.mult, mybir.AluOpType.max)
                            c = qt * NCQ + pcol[qt]
                            pcol[qt] += 1
                            nc.vector.tensor_reduce(
                                sall[:, c:c + 1], q16[:].bitcast(bf16),
                                axis=mybir.AxisListType.XY,
                                op=mybir.AluOpType.add)
                        cur[qt] = t0 + 512

            for qt in range(_QT):
                if half[qt] is not None:
                    q16 = half[qt]
                    c = qt * NCQ + pcol[qt]
                    pcol[qt] += 1
                    nc.vector.tensor_reduce(
                        sall[:, c:c + 1], q16[:, 0, :].bitcast(bf16),
                        axis=mybir.AxisListType.X, op=mybir.AluOpType.add)

            fin = spool.tile([128, _QT], f32)
            for qt in range(_QT):
                nc.vector.tensor_reduce(
                    fin[:, qt:qt + 1], sall[:, qt * NCQ:qt * NCQ + pcol[qt]],
                    axis=mybir.AxisListType.X, op=mybir.AluOpType.add)
            nc.sync.dma_start(out_d[:], fin[:])

    nc.compile()
    return nc


def _get_program(n_trains: int):
    if n_trains not in _prog_cache:
        _prog_cache[n_trains] = _build_program(n_trains)
    return _prog_cache[n_trains]


def _prep_inputs(X, X_train, sample_weight):
    X = np.ascontiguousarray(np.asarray(X, dtype=np.float32))
    Y = np.ascontiguousarray(np.asarray(X_train, dtype=np.float32))
    w = np.ascontiguousarray(np.asarray(sample_weight, dtype=np.float32))
    n = Y.shape[0]

    w64 = w.astype(np.float64)
    b64 = np.log(np.maximum(w64, 1e-300)) - 0.5 * np.sum(
        Y.astype(np.float64) ** 2, axis=1)
    b64 = np.clip(b64, -35.0, None)
    cb64 = (_C1 * b64 + _C2B / 32.0) / 4.0
    bhi = cb64.astype(np.float32).astype(_BF16)
    blo = (cb64 - bhi.astype(np.float64)).astype(np.float32).astype(_BF16)

    yext = np.empty((_K, n), dtype=_BF16)
    yext[0:32] = Y.astype(_BF16).T
    yext[32] = bhi
    yext[33] = blo

    const = 0.5 * _D * np.log(2.0 * np.pi) + np.log(np.sum(w64))
    xsq = np.sum(X.astype(np.float64) ** 2, axis=1)
    dv_all = (0.5 * xsq + const).astype(np.float32)

    in_maps = []
    dvs = []
    for c in range(_NCORES):
        sl = slice(c * _QSHARD, (c + 1) * _QSHARD)
        xq = X[sl]
        xext = np.empty((_K, _QSHARD), dtype=_BF16)
        xext[0:32] = (_C1 * xq.astype(np.float64)).astype(_BF16).T
        xext[32] = np.full(_QSHARD, 4.0, dtype=_BF16)
        xext[33] = np.full(_QSHARD, 4.0, dtype=_BF16)
        dv = np.ascontiguousarray(dv_all[sl].reshape(_QT, 128).T)
        in_maps.append({"yext": yext, "xext": xext})
        dvs.append(dv_all[sl].astype(np.float64))
    return in_maps, dvs


def _gather(results, dvs):
    out = np.empty(_Q, dtype=np.float32)
    for c in range(_NCORES):
        tot = results[c]["out"].T.reshape(_QSHARD).astype(np.float64)
        out[c * _QSHARD:(c + 1) * _QSHARD] = np.log(tot) - dvs[c]
    return out


def kernel(X, X_train, sample_weight, _want_timing=False):
    from concourse.bass_utils import run_bass_kernel_spmd

    nc = _get_program(_N)
    in_maps, dvs = _prep_inputs(X, X_train, sample_weight)
    kres = run_bass_kernel_spmd(
        nc, in_maps, core_ids=list(range(_NCORES)),
        trace=bool(_want_timing),
    )
    out = _gather(kres.results, dvs)
    if _want_timing:
        return out, kres
    return out


# revision 37
# speedup vs baseline: 1.0302x; 1.0099x over previous
"""Gaussian KDE (brute-force, bandwidth^2 = 1) on 8 Trainium2 NeuronCores.

Math:
    out_i = log( sum_j w_j * exp(-||x_i - y_j||^2/2) ) - (d/2) log(2pi) - log(sum_j w_j)
          = log( sum_j exp(x_i . y_j + b_j) ) - ||x_i||^2/2 - consts
    with b_j = log(w_j) - ||y_j||^2/2.

Queries sharded 8 ways (512/core, 4 PSUM-partition tiles). Per core:
    - scores: K=35 bf16 matmuls, stationary = query tile [35, 128], moving =
      train slices. Operands are pre-scaled so PSUM holds C1*s + C2b/32
      (C1 = 4/ln2, C2b the bf16 Schraudolph bias): x rows = C1*x dims plus
      three 4.0 rows; y rows = y dims + (C1/4)*b hi + lo + C2b/128 row.
      K=35 <= 64, so consecutive matmuls alternate PE row groups via
      tile_position (0,0)/(64,0) and run pairwise-concurrently.
    - exp+sum, two unit types balanced by a credit scheduler across engines:
      * ACT unit [128, 1536] (3 banks): table-exp in place with
        scale=1/C1, bias=-C2b/(32*C1), free-dim sum fused via accum_out.
      * DVE unit [128, 512] (1 bank): tensor_scalar (mult 32, max 0) whose
        int16 result IS the bf16 bit pattern of exp (Schraudolph; the max
        makes int16 wrap impossible), then tensor_reduce of the bitcast.
    - final: per query tile reduce partials, ln, subtract per-query const.
"""

import numpy as np
import ml_dtypes

_Q, _N, _D = 4096, 65536, 32
_NCORES = 8
_QSHARD = _Q // _NCORES          # 512 queries per core
_K = 34                          # 32 dims + bias hi/lo (incl C2 const)
_QT = 4                          # query tiles per core

_BF16 = ml_dtypes.bfloat16

_C1 = 4.0 / float(np.log(2.0))


def _c2b():
    f = (np.arange(100000, dtype=np.float64) + 0.5) / 100000.0
    m0 = np.mean((1.0 + f) * 2.0 ** (-f))
    m1 = np.mean(2.0 ** (-f))
    delta = (m0 - 1.0) / m1
    return float(127 * 128 - delta * 128)


_C2B = _c2b()

# per query tile: trains covered by ACT units (1536 each) and DVE units (512)
_NA = 30                         # 30 * 1536 = 46080
_ND = 38                         # 38 * 512  = 19456 ; total 65536
_ACT_NS = 1660.0                 # measured per ACT unit
_DVE_NS = 1294.0                 # measured per DVE unit (incl half reduce)

_prog_cache: dict = {}


def _unit_schedule():
    """Credit-scheduled unit type sequence (shared by all 4 query tiles)."""
    seq = []
    na, nd, ta, td = 0, 0, 0.0, -1300.0   # one D unit head start
    while na < _NA or nd < _ND:
        if nd >= _ND or (na < _NA and ta <= td):
            seq.append('A')
            na += 1
            ta += _ACT_NS
        else:
            seq.append('D')
            nd += 1
            td += _DVE_NS
    return seq


def _build_program(n_trains: int):
    import concourse.bass as bass
    import concourse.tile as tile
    from concourse import bacc, mybir

    f32 = mybir.dt.float32
    bf16 = mybir.dt.bfloat16
    i16 = mybir.dt.int16

    nc = bacc.Bacc("TRN2", target_bir_lowering=False, debug=False,
                   num_devices=_NCORES)

    y_d = nc.dram_tensor("yext", [_K, n_trains], bf16, kind="ExternalInput")
    x_d = nc.dram_tensor("xext", [_K, _QSHARD], bf16, kind="ExternalInput")
    out_d = nc.dram_tensor("out", [128, _QT], f32, kind="ExternalOutput")

    seq = _unit_schedule()

    with tile.TileContext(nc) as tc:
        with (
            tc.tile_pool(name="const", bufs=1) as cpool,
            tc.tile_pool(name="q16", bufs=8) as qpool,
            tc.tile_pool(name="small", bufs=2) as spool,
            tc.tile_pool(name="ps", bufs=1, space="PSUM") as ppool,
        ):
            xsb = cpool.tile([128, _QSHARD], bf16)
            nc.sync.dma_start(xsb[0:_K, :], x_d[:])
            nc.sync.dma_start(xsb[64:64 + _K, :], x_d[:])
            bias_sb = cpool.tile([128, 1], f32)
            nc.vector.memset(bias_sb[:], -_C2B / (32.0 * _C1))

            # y resident in SBUF, both row-group strips, 8 DMA pieces each
            ysb = cpool.tile([128, n_trains], bf16)
            pieces = [1024, 1024] + [2048] * 31
            off = 0
            for w in pieces:
                nc.sync.dma_start(ysb[0:_K, off:off + w],
                                  y_d[:, off:off + w])
                nc.sync.dma_start(ysb[64:64 + _K, off:off + w],
                                  y_d[:, off:off + w])
                off += w

            NCQ = 50
            sall = cpool.tile([128, NCQ * _QT], f32)

            # PSUM: A units double-buffered at [0:1536],[1536:3072];
            # D units at [3072:3584],[3584:4096]
            ps = ppool.tile([128, 8 * 512], f32)

            rg_par = [0]

            def score_mm(qt, dst, t0, width):
                for j in range(width // 512):
                    rg = 64 * (rg_par[0] & 1)
                    rg_par[0] += 1
                    nc.tensor.matmul(
                        out=ps[:, dst + j * 512: dst + (j + 1) * 512],
                        lhsT=xsb[rg:rg + _K, qt * 128:(qt + 1) * 128],
                        rhs=ysb[rg:rg + _K, t0 + j * 512: t0 + (j + 1) * 512],
                        start=True, stop=True,
                        tile_position=(rg, 0),
                    )

            pcol = [0] * _QT
            gen = {'A': 0, 'D': 0}
            cur = [0] * _QT
            half = [None] * _QT       # pending first half of a D pair
            # per-qt unit targets: mixed ratios balance ACT vs DVE closer
            tgt = [(30, 38), (30, 38), (29, 41), (29, 41)]
            na = [0] * _QT
            nd = [0] * _QT
            ta = [0.0] * _QT
            td = [-1300.0] * _QT
            while any(na[q] < tgt[q][0] or nd[q] < tgt[q][1]
                      for q in range(_QT)):
                for qt in range(_QT):
                    NAq, NDq = tgt[qt]
                    if na[qt] >= NAq and nd[qt] >= NDq:
                        continue
                    if nd[qt] >= NDq or (na[qt] < NAq and ta[qt] <= td[qt]):
                        typ = 'A'
                        na[qt] += 1
                        ta[qt] += _ACT_NS
                    else:
                        typ = 'D'
                        nd[qt] += 1
                        td[qt] += _DVE_NS
                    t0 = cur[qt]
                    if typ == 'A':
                        dst = 1536 * (gen['A'] & 1)
                        gen['A'] += 1
                        score_mm(qt, dst, t0, 1536)
                        c = qt * NCQ + pcol[qt]
                        pcol[qt] += 1
                        nc.scalar.activation(
                            ps[:, dst:dst + 1536], ps[:, dst:dst + 1536],
                            mybir.ActivationFunctionType.Exp,
                            bias=bias_sb[:], scale=1.0 / _C1,
                            accum_out=sall[:, c:c + 1])
                        cur[qt] = t0 + 1536
                    else:
                        dst = 3072 + 512 * (gen['D'] & 1)
                        gen['D'] += 1
                        score_mm(qt, dst, t0, 512)
                        if half[qt] is None:
                            q16 = qpool.tile([128, 2, 512], i16)
                            nc.vector.tensor_scalar(
                                q16[:, 0, :], ps[:, dst:dst + 512], 32.0, 0.0,
                                mybir.AluOpType.mult, mybir.AluOpType.max)
                            half[qt] = q16
                        else:
                            q16 = half[qt]
                            half[qt] = None
                            nc.vector.tensor_scalar(
                                q16[:, 1, :], ps[:, dst:dst + 512], 32.0, 0.0,
                                mybir.AluOpType.mult, mybir.AluOpType.max)
                            c = qt * NCQ + pcol[qt]
                            pcol[qt] += 1
                            nc.vector.tensor_reduce(
                                sall[:, c:c + 1], q16[:].bitcast(bf16),
                                axis=mybir.AxisListType.XY,
                                op=mybir.AluOpType.add)
                        cur[qt] = t0 + 512

            for qt in range(_QT):
                if half[qt] is not None:
                    q16 = half[qt]
                    c = qt * NCQ + pcol[qt]
                    pcol[qt] += 1
                    nc.vector.tensor_reduce(
                        sall[:, c:c + 1], q16[:, 0, :].bitcast(bf16),
                        axis=mybir.AxisListType.X, op=mybir.AluOpType.add)

            fin = spool.tile([128, _QT], f32)
            for qt in range(_QT):
                nc.vector.tensor_reduce(
                    fin[:, qt:qt + 1], sall[:, qt * NCQ:qt * NCQ + pcol[qt]],
                    axis=mybir.AxisListType.X, op=mybir.AluOpType.add)
            nc.sync.dma_start(out_d[:], fin[:])

    nc.compile()
    return nc


def _get_program(n_trains: int):
    if n_trains not in _prog_cache:
        _prog_cache[n_trains] = _build_program(n_trains)
    return _prog_cache[n_trains]


def _prep_inputs(X, X_train, sample_weight):
    X = np.ascontiguousarray(np.asarray(X, dtype=np.float32))
    Y = np.ascontiguousarray(np.asarray(X_train, dtype=np.float32))
    w = np.ascontiguousarray(np.asarray(sample_weight, dtype=np.float32))
    n = Y.shape[0]

    w64 = w.astype(np.float64)
    b64 = np.log(np.maximum(w64, 1e-300)) - 0.5 * np.sum(
        Y.astype(np.float64) ** 2, axis=1)
    b64 = np.clip(b64, -35.0, None)
    cb64 = (_C1 * b64 + _C2B / 32.0) / 4.0
    bhi = cb64.astype(np.float32).astype(_BF16)
    blo = (cb64 - bhi.astype(np.float64)).astype(np.float32).astype(_BF16)

    yext = np.empty((_K, n), dtype=_BF16)
    yext[0:32] = Y.astype(_BF16).T
    yext[32] = bhi
    yext[33] = blo

    const = 0.5 * _D * np.log(2.0 * np.pi) + np.log(np.sum(w64))
    xsq = np.sum(X.astype(np.float64) ** 2, axis=1)
    dv_all = (0.5 * xsq + const).astype(np.float32)

    in_maps = []
    dvs = []
    for c in range(_NCORES):
        sl = slice(c * _QSHARD, (c + 1) * _QSHARD)
        xq = X[sl]
        xext = np.empty((_K, _QSHARD), dtype=_BF16)
        xext[0:32] = (_C1 * xq.astype(np.float64)).astype(_BF16).T
        xext[32] = np.full(_QSHARD, 4.0, dtype=_BF16)
        xext[33] = np.full(_QSHARD, 4.0, dtype=_BF16)
        dv = np.ascontiguousarray(dv_all[sl].reshape(_QT, 128).T)
        in_maps.append({"yext": yext, "xext": xext})
        dvs.append(dv_all[sl].astype(np.float64))
    return in_maps, dvs


def _gather(results, dvs):
    out = np.empty(_Q, dtype=np.float32)
    for c in range(_NCORES):
        tot = results[c]["out"].T.reshape(_QSHARD).astype(np.float64)
        out[c * _QSHARD:(c + 1) * _QSHARD] = np.log(tot) - dvs[c]
    return out


def kernel(X, X_train, sample_weight, _want_timing=False):
    from concourse.bass_utils import run_bass_kernel_spmd

    nc = _get_program(_N)
    in_maps, dvs = _prep_inputs(X, X_train, sample_weight)
    kres = run_bass_kernel_spmd(
        nc, in_maps, core_ids=list(range(_NCORES)),
        trace=bool(_want_timing),
    )
    out = _gather(kres.results, dvs)
    if _want_timing:
        return out, kres
    return out
